# revision 1
# baseline (speedup 1.0000x reference)
"""GATv2 (2-layer, heads=1) on 8 trn2 NeuronCores — self-contained kernel.

Strategy (graph/data parallel, per the sharding hint):
- Nodes are dealt to 8 cores by in-degree rank; each core owns SHARD node
  slots (rows). Within a core, nodes are sorted by (deg, m0-m2) and grouped
  into chunks of 128 lanes.
- Edge features are gathered into a padded grid [lane-partition, slot-block]
  via InstDMAGatherAnt (int16 indices). Three overlapping table windows
  (base 0 / W1B / W2B) cover all rows despite the int16 range; edges in
  window overlaps are assigned to whichever window's slot range has room.
- The feature table holds rows [w*perm(x@Wl) | w*perm(x@Wr)] (512B bf16),
  w = |att|, perm = positive-att dims first. Then
    logit_e = att . leaky_relu(xl[src]+xr[dst]+bl+br)
            = sum_{d<Pp} lrelu(v'_d) - sum_{d>=Pp} lrelu(v'_d)
  with v' = gathered_wxl + (xr'[dst] + w*perm(bl+br)): two structured
  free-dim reduces, no per-edge matvec.
- Row softmax over slots; out = sum_s a_s*xl_s computed as
  sum_s a_s*v'_s (identity-lhsT matmuls accumulated in PSUM) minus the
  linear correction (sum a)*(xr'+br'). Layer outputs stay in w*perm space;
  the host folds the inverse into the next layer's weights / final output.
- Layer 2 exchanges activations with one AllGather of transposed shards.
"""

import numpy as np

SLOPE = 0.2
CORES = 8
GRP = 4  # chunks per gather-call group


class Cfg:
    def __init__(self, N, E, SHARD, W0E, W1B, W1E, W2B):
        self.N, self.E, self.SHARD = N, E, SHARD
        self.NCHUNK = SHARD // 128
        self.NPOS = CORES * SHARD
        self.W0E, self.W1B, self.W1E, self.W2B = W0E, W1B, W1E, W2B
        self.WB = [0, W1B, W2B]
        self.WE = [W0E, W1E, self.NPOS]
        # zero rows: a pad lane inside each window (cores 0, mid, 7)
        npercore = [N // CORES + (1 if c < N % CORES else 0) for c in range(CORES)]
        self.ZROW = []
        for w in range(3):
            z = None
            for c in range(CORES):
                r = c * SHARD + npercore[c]  # first pad row of core c
                if self.WB[w] <= r < self.WE[w] and npercore[c] < SHARD:
                    z = r
                    break
            assert z is not None, f"no pad row inside window {w}"
            self.ZROW.append(z)


FULL = Cfg(N=50000, E=800000, SHARD=6272, W0E=32768, W1B=8704, W1E=41472, W2B=17408)
IN = 128
D = 128
ROWE = 256  # bf16 elems per table row (512 B)


# ----------------------------------------------------------------------------
# host-side graph preprocessing
# ----------------------------------------------------------------------------


def preprocess(edge_index, cfg=FULL):
    src = np.asarray(edge_index[0], dtype=np.int64)
    dst = np.asarray(edge_index[1], dtype=np.int64)
    N, SHARD, NCHUNK, NPOS = cfg.N, cfg.SHARD, cfg.NCHUNK, cfg.NPOS
    deg = np.bincount(dst, minlength=N)

    order = np.argsort(-deg, kind="stable")
    core_of = np.empty(N, dtype=np.int64)
    core_of[order] = np.arange(N) % CORES

    def assign_rows(key1, key2):
        row_of = np.empty(N, dtype=np.int64)
        nodemap = np.full((CORES, SHARD), -1, dtype=np.int64)
        for c in range(CORES):
            nodes = np.where(core_of == c)[0]
            k = np.lexsort((-key2[nodes], -key1[nodes]))
            nodes = nodes[k]
            row_of[nodes] = c * SHARD + np.arange(len(nodes))
            nodemap[c, : len(nodes)] = nodes
        return row_of, nodemap

    row_of, nodemap = assign_rows(deg, np.zeros(N, dtype=np.int64))
    for _ in range(2):
        src_rows = row_of[src]
        m0 = np.bincount(dst[src_rows < cfg.W1B], minlength=N)
        m2 = np.bincount(dst[src_rows >= cfg.W1E], minlength=N)
        row_of, nodemap = assign_rows(deg, m0 - m2)
    src_rows = row_of[src]
    dst_rows = row_of[dst]

    eorder = np.lexsort((src_rows, dst_rows))
    s_sorted = src_rows[eorder]
    d_sorted = dst_rows[eorder]
    starts = np.searchsorted(d_sorted, np.arange(NPOS))
    ends = np.searchsorted(d_sorted, np.arange(NPOS) + 1)

    lanes_chunk = (np.arange(NPOS) % SHARD) // 128

    # per-chunk slot profile (K0, K1, K2) via small 2D search
    K0 = np.zeros(NCHUNK, dtype=np.int64)
    K1 = np.zeros(NCHUNK, dtype=np.int64)
    K2 = np.zeros(NCHUNK, dtype=np.int64)
    for c in range(NCHUNK):
        lanes = np.where(lanes_chunk == c)[0]
        lists = [s_sorted[starts[p] : ends[p]] for p in lanes]
        degl = np.array([len(sl) for sl in lists])
        n_lt_w0e = np.array([np.searchsorted(sl, cfg.W0E) for sl in lists])
        n_lt_w1e = np.array([np.searchsorted(sl, cfg.W1E) for sl in lists])
        n_lt_w1b = np.array([np.searchsorted(sl, cfg.W1B) for sl in lists])
        n_lt_w2b = np.array([np.searchsorted(sl, cfg.W2B) for sl in lists])
        best = None
        for k0 in range(max(1, int(n_lt_w1b.max())), int(n_lt_w0e.max()) + 2):
            t0 = np.minimum(n_lt_w0e, k0)
            m1 = np.maximum(n_lt_w2b - t0, 0)
            for k1 in range(max(1, int(m1.max())), int((n_lt_w1e - t0).max()) + 2):
                t1 = np.minimum(n_lt_w1e - t0, k1)
                k2 = max(1, int((degl - t0 - t1).max()))
                if best is None or k0 + k1 + k2 < best[0]:
                    best = (k0 + k1 + k2, k0, k1, k2)
        K0[c], K1[c], K2[c] = best[1], best[2], best[3]

    K = K0 + K1 + K2
    koff = np.concatenate([[0], np.cumsum(K)])
    TK = int(koff[-1])
    # idx grids per window (values are window-relative rows); masks
    idx_rows = np.zeros((CORES, TK, 128), dtype=np.int64)
    mask = np.zeros((CORES, TK, 128), dtype=np.float32)
    for c in range(NCHUNK):
        b = koff[c]
        idx_rows[:, b : b + K0[c], :] = cfg.ZROW[0]
        idx_rows[:, b + K0[c] : b + K0[c] + K1[c], :] = cfg.ZROW[1]
        idx_rows[:, b + K0[c] + K1[c] : koff[c + 1], :] = cfg.ZROW[2]
    for p in range(NPOS):
        e0, e1 = starts[p], ends[p]
        if e1 == e0:
            continue
        core, pos = p // SHARD, p % SHARD
        c, lane = pos // 128, pos % 128
        sl = s_sorted[e0:e1]
        t0 = min(int(np.searchsorted(sl, cfg.W0E)), int(K0[c]))
        t1 = min(int(np.searchsorted(sl, cfg.W1E)) - t0, int(K1[c]))
        t2 = len(sl) - t0 - t1
        assert t2 <= K2[c]
        b = koff[c]
        idx_rows[core, b : b + t0, lane] = sl[:t0]
        mask[core, b : b + t0, lane] = 1.0
        b1 = koff[c] + K0[c]
        idx_rows[core, b1 : b1 + t1, lane] = sl[t0 : t0 + t1]
        mask[core, b1 : b1 + t1, lane] = 1.0
        b2 = koff[c] + K0[c] + K1[c]
        idx_rows[core, b2 : b2 + t2, lane] = sl[t0 + t1 :]
        mask[core, b2 : b2 + t2, lane] = 1.0
    # window-range sanity
    for c in range(NCHUNK):
        b = koff[c]
        assert (idx_rows[:, b : b + K0[c], :] < cfg.W0E).all()
        h1 = idx_rows[:, b + K0[c] : b + K0[c] + K1[c], :]
        assert (h1 >= cfg.W1B).all() and (h1 < cfg.W1E).all()
        h2 = idx_rows[:, b + K0[c] + K1[c] : koff[c + 1], :]
        assert (h2 >= cfg.W2B).all()

    return dict(
        row_of=row_of, nodemap=nodemap, K0=K0, K1=K1, K2=K2, K=K, koff=koff,
        idx_rows=idx_rows, mask=mask, cfg=cfg,
    )


def transform_weights(Wl, bl, Wr, br, att, bias, in_perm=None, in_w=None):
    Wl = np.asarray(Wl, np.float64)
    Wr = np.asarray(Wr, np.float64)
    bl = np.asarray(bl, np.float64)
    br = np.asarray(br, np.float64)
    att = np.asarray(att, np.float64)
    bias = np.asarray(bias, np.float64)
    if in_perm is not None:
        scale = 1.0 / in_w[in_perm]
        Wl = Wl[in_perm, :] * scale[:, None]
        Wr = Wr[in_perm, :] * scale[:, None]
    w = np.abs(att)
    pos = np.where(att >= 0)[0]
    neg = np.where(att < 0)[0]
    perm = np.concatenate([pos, neg])

    def colT(W):
        return W[:, perm] * w[perm][None, :]

    def vecT(v):
        return (w * v)[perm]

    return dict(
        Wc=np.concatenate([colT(Wl), colT(Wr)], axis=1),
        Bp=vecT(bl + br), blp=vecT(bl), biasp=vecT(bias),
        perm=perm, w=w, Pp=len(pos),
    )


def host_transforms(params):
    t1 = transform_weights(
        params["Wl1"], params["bl1"], params["Wr1"], params["br1"],
        params["att1"], params["bias1"],
    )
    t2 = transform_weights(
        params["Wl2"], params["bl2"], params["Wr2"], params["br2"],
        params["att2"], params["bias2"], in_perm=t1["perm"], in_w=t1["w"],
    )
    return t1, t2


# ----------------------------------------------------------------------------
# numpy emulator of the device algorithm (validation)
# ----------------------------------------------------------------------------


def emulate(node_fts, params, pp):
    cfg = pp["cfg"]
    t1, t2 = host_transforms(params)
    nodemap = pp["nodemap"]
    nm = nodemap.reshape(-1)
    valid = nm >= 0
    x_rows = np.zeros((cfg.NPOS, IN), dtype=np.float64)
    x_rows[valid] = np.asarray(node_fts, np.float64)[nm[valid]]

    def layer(x_rows, t):
        table = x_rows @ t["Wc"]
        wxl, wxr = table[:, :128], table[:, 128:]
        koff, K = pp["koff"], pp["K"]
        idx_rows, mask = pp["idx_rows"], pp["mask"]
        out = np.zeros((cfg.NPOS, 128), dtype=np.float64)
        Pp = t["Pp"]
        for core in range(CORES):
            for c in range(cfg.NCHUNK):
                b, e = koff[c], koff[c + 1]
                rows = core * cfg.SHARD + c * 128 + np.arange(128)
                idx = idx_rows[core, b:e, :]
                m = mask[core, b:e, :]
                xrb = wxr[rows] + t["Bp"][None, :]
                v = wxl[idx] + xrb[None, :, :]  # [K, lane, d]
                u = np.where(v > 0, v, SLOPE * v)
                logit = u[:, :, :Pp].sum(-1) - u[:, :, Pp:].sum(-1)
                a = np.exp(np.minimum(logit, 60.0)) * m
                s = a.sum(0)
                r = 1.0 / np.maximum(s, 1e-16)
                a_n = a * r[None, :]
                S1 = s * r
                psum = np.einsum("kl,kld->ld", a_n, wxl[idx])
                out[rows] = psum + S1[:, None] * t["blp"][None, :] + t["biasp"][None, :]
        return np.maximum(out, 0.0)

    x2 = layer(x_rows, t1)
    out2 = layer(x2, t2)
    un = out2 / t2["w"][t2["perm"]][None, :]
    full = np.zeros((cfg.N, D), dtype=np.float64)
    full[nm[valid]] = un[valid][:, np.argsort(t2["perm"])]
    return full


# ----------------------------------------------------------------------------
# device program
# ----------------------------------------------------------------------------


def wrap_idx(flat):
    """flat int idx list (len % 16 == 0) -> [128, n/16] int16 (16-wrap,
    replicated across the 8 Q7 core blocks)."""
    n = flat.shape[0]
    w = flat.reshape(n // 16, 16).T.astype(np.int16)
    return np.tile(w, (8, 1))


def make_groups(K):
    """Greedy chunk groups capped by total slot-blocks (SBUF) and count."""
    CAP = 44
    groups = []
    cur = []
    tot = 0
    for c in range(len(K)):
        k = int(K[c])
        if cur and (tot + k > CAP or len(cur) >= GRP):
            groups.append(cur)
            cur, tot = [], 0
        cur.append(c)
        tot += k
    if cur:
        groups.append(cur)
    return groups


def build_program(pp, Pp1, Pp2, rep=1, debug_out=False, only_layer1=False):
    import concourse.bass as bass
    import concourse.mybir as mybir
    import concourse.tile as tile
    from concourse import bacc

    cfg = pp["cfg"]
    fp32, bf16, i16 = mybir.dt.float32, mybir.dt.bfloat16, mybir.dt.int16
    K0, K1, K2, K, koff = pp["K0"], pp["K1"], pp["K2"], pp["K"], pp["koff"]
    NCHUNK, SHARD, NPOS = cfg.NCHUNK, cfg.SHARD, cfg.NPOS
    TK = int(koff[-1])
    NT = NPOS // 128  # node tiles
    XRC = SHARD // 16  # xr idx cols per window

    # per-group gather call sizes per window
    groups = make_groups(K)
    gsz = []  # (n0, n1, n2) idx counts per group
    for chs in groups:
        gsz.append(
            (
                int(sum(K0[c] for c in chs)) * 128,
                int(sum(K1[c] for c in chs)) * 128,
                int(sum(K2[c] for c in chs)) * 128,
            )
        )
    i0cols = sum(n0 // 16 for n0, _, _ in gsz)
    i1cols = sum(n1 // 16 for _, n1, _ in gsz)
    i2cols = sum(n2 // 16 for _, _, n2 in gsz)

    nc = bacc.Bacc(
        "TRN2", target_bir_lowering=False, debug=False, num_devices=CORES,
        num_swdge_queues=4,
    )
    # inputs
    xT = nc.dram_tensor("xT", [128, NPOS], bf16, kind="ExternalInput").ap()
    W1c = nc.dram_tensor("W1c", [128, 256], bf16, kind="ExternalInput").ap()
    W2c = nc.dram_tensor("W2c", [128, 256], bf16, kind="ExternalInput").ap()
    # consts: [identity | B1 | br1 | bias1 | B2 | br2 | bias2] each [128,128]
    cons = nc.dram_tensor("cons", [128, 7 * 128], bf16, kind="ExternalInput").ap()
    idx0 = nc.dram_tensor("idx0", [128, i0cols], i16, kind="ExternalInput").ap()
    idx1 = nc.dram_tensor("idx1", [128, i1cols], i16, kind="ExternalInput").ap()
    idx2 = nc.dram_tensor("idx2", [128, i2cols], i16, kind="ExternalInput").ap()
    xidx = nc.dram_tensor("xidx", [128, 3 * XRC], i16, kind="ExternalInput").ap()
    maskt = nc.dram_tensor("maskt", [128, TK], bf16, kind="ExternalInput").ap()
    # internal / outputs
    table = nc.dram_tensor("table", [NPOS, ROWE], bf16)
    x2s = nc.dram_tensor("x2s", [SHARD, 128], bf16)
    x2t_d = nc.dram_tensor("x2t_d", [128, SHARD], bf16)
    ag = nc.dram_tensor("ag", [CORES * 128, SHARD], bf16, addr_space="Shared")
    out_ext = nc.dram_tensor("out", [SHARD, 128], fp32, kind="ExternalOutput").ap()
    if debug_out:
        dbg_table = nc.dram_tensor(
            "dbg_table", [NPOS, ROWE], bf16, kind="ExternalOutput"
        ).ap()
        dbg_x2 = nc.dram_tensor(
            "dbg_x2", [SHARD, 128], bf16, kind="ExternalOutput"
        ).ap()

    with tile.TileContext(nc) as tc:
        with (
            tc.tile_pool(name="res", bufs=1) as res,
            tc.tile_pool(name="xr3", bufs=1) as xr3,
            tc.tile_pool(name="gsb", bufs=2) as gsb,
            tc.tile_pool(name="csb", bufs=3) as csb,
            tc.tile_pool(name="nsb", bufs=3) as nsb,
            tc.tile_pool(name="ps", bufs=2, space="PSUM") as ps,
            tc.tile_pool(name="ps2", bufs=2, space="PSUM") as ps2,
        ):
            # resident loads
            i0_sb = res.tile([128, i0cols], i16, tag="i0")
            i1_sb = res.tile([128, i1cols], i16, tag="i1")
            i2_sb = res.tile([128, i2cols], i16, tag="i2")
            xi_sb = res.tile([128, 3 * XRC], i16, tag="xi")
            mk_sb = res.tile([128, TK], bf16, tag="mk")
            co_sb = res.tile([128, 7 * 128], bf16, tag="co")
            w1_sb = res.tile([128, 256], bf16, tag="w1")
            w2_sb = res.tile([128, 256], bf16, tag="w2")
            nc.sync.dma_start(out=i0_sb[:], in_=idx0[:])
            nc.sync.dma_start(out=i1_sb[:], in_=idx1[:])
            nc.sync.dma_start(out=i2_sb[:], in_=idx2[:])
            nc.sync.dma_start(out=xi_sb[:], in_=xidx[:])
            nc.sync.dma_start(out=mk_sb[:], in_=maskt[:])
            nc.sync.dma_start(out=co_sb[:], in_=cons[:])
            nc.sync.dma_start(out=w1_sb[:], in_=W1c[:])
            nc.sync.dma_start(out=w2_sb[:], in_=W2c[:])
            ident = co_sb[:, 0:128]

            def node_phase(layer):
                w_sb = w1_sb if layer == 1 else w2_sb
                for q in range((NT + 3) // 4):
                    tiles = [t for t in range(q * 4, min(q * 4 + 4, NT))]
                    nq = len(tiles)
                    lhs = nsb.tile([128, nq * 128], bf16, tag="lhs")
                    if layer == 1:
                        nc.sync.dma_start(
                            out=lhs[:],
                            in_=xT[:, tiles[0] * 128 : tiles[0] * 128 + nq * 128],
                        )
                    else:
                        # lhsT tiles from ag: rows of owner core, cols in-shard
                        for j, t in enumerate(tiles):
                            o = (t * 128) // SHARD
                            p0 = t * 128 - o * SHARD
                            nc.sync.dma_start(
                                out=lhs[:, j * 128 : (j + 1) * 128],
                                in_=ag[o * 128 : (o + 1) * 128, p0 : p0 + 128],
                            )
                    pt = ps.tile([128, nq * 256], fp32, tag="np")
                    for j in range(nq):
                        nc.tensor.matmul(
                            pt[:, j * 256 : (j + 1) * 256],
                            lhsT=lhs[:, j * 128 : (j + 1) * 128],
                            rhs=w_sb[:],
                            start=True,
                            stop=True,
                        )
                    rows = nsb.tile([128, nq * 256], bf16, tag="rows")
                    if q % 2 == 0:
                        nc.vector.tensor_copy(rows[:], pt[:])
                    else:
                        nc.scalar.copy(rows[:], pt[:])
                    # SBUF [128p, nq, 256] -> DRAM rows [nq*128, 256]
                    dst = table[tiles[0] * 128 : tiles[0] * 128 + nq * 128].rearrange(
                        "(j p) c -> p j c", p=128
                    )
                    nc.sync.dma_start(out=dst, in_=rows[:].rearrange("p (j c) -> p j c", c=256))

            def edge_phase(layer):
                Pp = Pp1 if layer == 1 else Pp2
                cb = 1 if layer == 1 else 4  # consts block base
                B_rep = co_sb[:, cb * 128 : (cb + 1) * 128]
                bl_rep = co_sb[:, (cb + 1) * 128 : (cb + 2) * 128]
                bias_rep = co_sb[:, (cb + 2) * 128 : (cb + 3) * 128]

                # xr gather: 3 windows; 2 SBUF slots (reuse the scratch one)
                xr_all = xr3.tile([128, (SHARD // 128) * ROWE], bf16, tag="xra")
                nc.gpsimd.dma_gather(
                    out_ap=xr_all[:].rearrange("p (b r) -> p b r", r=ROWE),
                    in_ap=table[cfg.WB[0] :, :],
                    idxs_ap=xi_sb[:, 0:XRC],
                    num_idxs=SHARD,
                    num_idxs_reg=SHARD,
                    elem_size=ROWE,
                    single_packet=False,
                    queue_num=0,
                )
                for w in (1, 2):
                    t = xr3.tile([128, (SHARD // 128) * ROWE], bf16, tag="xrs")
                    nc.gpsimd.dma_gather(
                        out_ap=t[:].rearrange("p (b r) -> p b r", r=ROWE),
                        in_ap=table[cfg.WB[w] :, :],
                        idxs_ap=xi_sb[:, w * XRC : (w + 1) * XRC],
                        num_idxs=SHARD,
                        num_idxs_reg=SHARD,
                        elem_size=ROWE,
                        single_packet=False,
                        queue_num=w,
                    )
                    nc.vector.tensor_tensor(
                        out=xr_all[:], in0=xr_all[:], in1=t[:], op=mybir.AluOpType.add
                    )

                o0 = o1 = o2 = 0
                for gi, chs in enumerate(groups):
                    n0, n1, n2 = gsz[gi]
                    kg = int(sum(K[c] for c in chs))
                    gt = gsb.tile([128, kg * ROWE], bf16, tag="g")
                    g3 = gt[:].rearrange("p (b r) -> p b r", r=ROWE)
                    # block ranges within the group tile per window
                    blk = 0
                    w_blk = []
                    for w, kw in ((0, K0), (1, K1), (2, K2)):
                        nblk = int(sum(kw[c] for c in chs))
                        w_blk.append((blk, nblk))
                        blk += nblk
                    for w, (isb, off, nn) in enumerate(
                        ((i0_sb, o0, n0), (i1_sb, o1, n1), (i2_sb, o2, n2))
                    ):
                        b0, nblk = w_blk[w]
                        nc.gpsimd.dma_gather(
                            out_ap=g3[:, b0 : b0 + nblk, :],
                            in_ap=table[cfg.WB[w] :, :],
                            idxs_ap=isb[:, off : off + nn // 16],
                            num_idxs=nn,
                            num_idxs_reg=nn,
                            elem_size=ROWE,
                            single_packet=False,
                            queue_num=(gi * 3 + w) % 4,
                        )
                    o0 += n0 // 16
                    o1 += n1 // 16
                    o2 += n2 // 16

                    for ci, c in enumerate(chs):
                        kc = int(K[c])
                        # chunk's blocks within group tile, per window
                        cblk = []
                        for w, kw in ((0, K0), (1, K1), (2, K2)):
                            b0 = w_blk[w][0] + int(sum(kw[cc] for cc in chs[:ci]))
                            cblk.append((b0, int(kw[c])))
                        xr_c = xr_all[:].rearrange("p (b r) -> p b r", r=ROWE)[
                            :, c, 128:256
                        ]
                        xrb = csb.tile([128, 128], bf16, tag="xrb")
                        nc.vector.tensor_tensor(
                            out=xrb[:], in0=xr_c, in1=B_rep, op=mybir.AluOpType.add
                        )
                        # v' = wxl + xrb per slot, into the slot's 2nd half
                        for b0, nb in cblk:
                            for b in range(b0, b0 + nb):
                                nc.vector.tensor_tensor(
                                    out=g3[:, b, 128:256],
                                    in0=g3[:, b, 0:128],
                                    in1=xrb[:],
                                    op=mybir.AluOpType.add,
                                )
                        # gather chunk's v' slot-slices view (3 window ranges
                        # are contiguous block runs; process each run)
                        ut = csb.tile([128, kc * 128], bf16, tag="u")
                        uoff = 0
                        runs = []
                        for b0, nb in cblk:
                            if nb == 0:
                                continue
                            runs.append((b0, nb, uoff))
                            uoff += nb
                        for b0, nb, uo in runs:
                            nc.scalar.activation(
                                out=ut[:].rearrange("p (b r) -> p b r", r=128)[
                                    :, uo : uo + nb, :
                                ],
                                in_=g3[:, b0 : b0 + nb, 128:256],
                                func=mybir.ActivationFunctionType.Prelu,
                                alpha=SLOPE,
                            )
                        u3 = ut[:].rearrange("p (b r) -> p b r", r=128)
                        lg = csb.tile([128, kc], fp32, tag="lg")
                        lgn = csb.tile([128, kc], fp32, tag="lgn")
                        nc.vector.tensor_reduce(
                            out=lg[:], in_=u3[:, :, 0:Pp], axis=mybir.AxisListType.X,
                            op=mybir.AluOpType.add,
                        )
                        if Pp < 128:
                            nc.vector.tensor_reduce(
                                out=lgn[:], in_=u3[:, :, Pp:128],
                                axis=mybir.AxisListType.X, op=mybir.AluOpType.add,
                            )
                            nc.vector.tensor_tensor(
                                out=lg[:], in0=lg[:], in1=lgn[:],
                                op=mybir.AluOpType.subtract,
                            )
                        nc.vector.tensor_scalar_min(lg[:], lg[:], 60.0)
                        av = csb.tile([128, kc], fp32, tag="av")
                        nc.scalar.activation(
                            out=av[:], in_=lg[:], func=mybir.ActivationFunctionType.Exp
                        )
                        nc.vector.tensor_tensor(
                            out=av[:], in0=av[:],
                            in1=mk_sb[:, koff[c] : koff[c] + kc],
                            op=mybir.AluOpType.mult,
                        )
                        sv = csb.tile([128, 4], fp32, tag="sv")
                        nc.vector.tensor_reduce(
                            out=sv[:, 0:1], in_=av[:], axis=mybir.AxisListType.X,
                            op=mybir.AluOpType.add,
                        )
                        nc.vector.tensor_scalar_max(sv[:, 1:2], sv[:, 0:1], 1e-16)
                        nc.vector.reciprocal(sv[:, 2:3], sv[:, 1:2])
                        nc.vector.tensor_scalar(
                            out=sv[:, 3:4], in0=sv[:, 0:1], scalar1=sv[:, 2:3],
                            scalar2=None, op0=mybir.AluOpType.mult,
                        )
                        an = csb.tile([128, kc], fp32, tag="an")
                        nc.vector.tensor_scalar(
                            out=an[:], in0=av[:], scalar1=sv[:, 2:3], scalar2=None,
                            op0=mybir.AluOpType.mult,
                        )
                        # out-path: psum += ident.T @ (a_n[s] * v'_s)
                        opsum = ps2.tile([128, 128], fp32, tag="op")
                        si = 0
                        for b0, nb, uo in runs:
                            for b in range(b0, b0 + nb):
                                avs = csb.tile([128, 128], bf16, tag="avs")
                                eng = nc.vector if (si % 2 == 0) else nc.scalar
                                if si % 2 == 0:
                                    nc.vector.tensor_scalar(
                                        out=avs[:], in0=g3[:, b, 0:128],
                                        scalar1=an[:, si : si + 1], scalar2=None,
                                        op0=mybir.AluOpType.mult,
                                    )
                                else:
                                    nc.scalar.activation(
                                        out=avs[:], in_=g3[:, b, 0:128],
                                        func=mybir.ActivationFunctionType.Copy,
                                        scale=an[:, si : si + 1],
                                    )
                                nc.tensor.matmul(
                                    opsum[:], lhsT=ident, rhs=avs[:],
                                    start=(si == 0), stop=(si == kc - 1),
                                )
                                si += 1
                        # of = psum + (S1*blp' + bias')
                        corr = csb.tile([128, 128], fp32, tag="corr")
                        nc.vector.tensor_scalar(
                            out=corr[:], in0=bl_rep, scalar1=sv[:, 3:4], scalar2=None,
                            op0=mybir.AluOpType.mult,
                        )
                        nc.vector.tensor_tensor(
                            out=corr[:], in0=corr[:], in1=bias_rep,
                            op=mybir.AluOpType.add,
                        )
                        of = csb.tile([128, 128], fp32, tag="of")
                        nc.vector.tensor_tensor(
                            out=of[:], in0=opsum[:], in1=corr[:],
                            op=mybir.AluOpType.add,
                        )
                        if layer == 1:
                            xrow = csb.tile([128, 128], bf16, tag="xrow")
                            nc.scalar.activation(
                                out=xrow[:], in_=of[:],
                                func=mybir.ActivationFunctionType.Relu,
                            )
                            nc.sync.dma_start(
                                out=x2s[c * 128 : (c + 1) * 128, :], in_=xrow[:]
                            )
                        else:
                            orow = csb.tile([128, 128], fp32, tag="orow")
                            nc.scalar.activation(
                                out=orow[:], in_=of[:],
                                func=mybir.ActivationFunctionType.Relu,
                            )
                            nc.sync.dma_start(
                                out=out_ext[c * 128 : (c + 1) * 128, :], in_=orow[:]
                            )

            for _ in range(rep):
                node_phase(1)
                edge_phase(1)
                if debug_out:
                    nc.sync.dma_start(out=dbg_table[:], in_=table[:])
                    nc.sync.dma_start(out=dbg_x2[:], in_=x2s[:])
                if only_layer1:
                    continue
                # exchange x2 (transposed shards -> AllGather)
                x2t = gsb.tile([128, SHARD], bf16, tag="x2t")
                nc.sync.dma_start(out=x2t[:], in_=x2s[:], transpose=True)
                nc.sync.dma_start(out=x2t_d[:], in_=x2t[:])
                nc.gpsimd.collective_compute(
                    "AllGather",
                    mybir.AluOpType.bypass,
                    replica_groups=[list(range(CORES))],
                    ins=[x2t_d[:]],
                    outs=[ag[:]],
                )
                node_phase(2)
                edge_phase(2)

    nc.compile()
    return nc


# ----------------------------------------------------------------------------
# host input packing + entry point
# ----------------------------------------------------------------------------


def make_inputs(node_fts, params, pp):
    import ml_dtypes

    bf = ml_dtypes.bfloat16
    cfg = pp["cfg"]
    t1, t2 = host_transforms(params)
    nodemap = pp["nodemap"]
    nm = nodemap.reshape(-1)
    valid = nm >= 0
    x_rows = np.zeros((cfg.NPOS, IN), dtype=np.float32)
    x_rows[valid] = np.asarray(node_fts, np.float32)[nm[valid]]
    xT = np.ascontiguousarray(x_rows.T).astype(bf)

    def consts(t):
        return [
            np.tile(np.asarray(v, np.float32).astype(bf)[None, :], (128, 1))
            for v in (t["Bp"], t["blp"], t["biasp"])
        ]

    cons = np.concatenate(
        [np.eye(128, dtype=np.float32).astype(bf)] + consts(t1) + consts(t2), axis=1
    )

    K0, K1, K2, koff = pp["K0"], pp["K1"], pp["K2"], pp["koff"]
    NCHUNK, SHARD = cfg.NCHUNK, cfg.SHARD
    groups = make_groups(pp["K"])
    idx_rows, mask = pp["idx_rows"], pp["mask"]

    in_maps = []
    for core in range(CORES):
        i0l, i1l, i2l = [], [], []
        for chs in groups:
            f0, f1, f2 = [], [], []
            for c in chs:
                b = koff[c]
                f0.append(idx_rows[core, b : b + K0[c], :] - cfg.WB[0])
                f1.append(
                    idx_rows[core, b + K0[c] : b + K0[c] + K1[c], :] - cfg.WB[1]
                )
                f2.append(idx_rows[core, b + K0[c] + K1[c] : koff[c + 1], :] - cfg.WB[2])
            i0l.append(wrap_idx(np.concatenate(f0).ravel()))
            i1l.append(wrap_idx(np.concatenate(f1).ravel()))
            i2l.append(wrap_idx(np.concatenate(f2).ravel()))
        xi = []
        own = core * SHARD + np.arange(SHARD)
        # each row is fetched by exactly one window (the first that covers it)
        wsel = np.full(SHARD, 2, dtype=np.int64)
        wsel[own < cfg.WE[1]] = 1
        wsel[own < cfg.WE[0]] = 0
        for w in range(3):
            vals = np.where(wsel == w, own - cfg.WB[w], cfg.ZROW[w] - cfg.WB[w])
            xi.append(wrap_idx(vals))
        in_maps.append(
            {
                "xT": xT,
                "W1c": np.asarray(t1["Wc"], np.float32).astype(bf),
                "W2c": np.asarray(t2["Wc"], np.float32).astype(bf),
                "cons": cons.astype(bf),
                "idx0": np.concatenate(i0l, axis=1),
                "idx1": np.concatenate(i1l, axis=1),
                "idx2": np.concatenate(i2l, axis=1),
                "xidx": np.concatenate(xi, axis=1),
                "maskt": np.tile(
                    mask[core].T.astype(bf)[None, :, :], (1, 1, 1)
                ).reshape(128, -1),
                # mask[core] is [TK, 128] -> need [128, TK]
            }
        )
        in_maps[-1]["maskt"] = np.ascontiguousarray(mask[core].T).astype(bf)
    return in_maps, (t1, t2)


def postprocess(results, pp, t2):
    cfg = pp["cfg"]
    nodemap = pp["nodemap"]
    out = np.zeros((cfg.N, D), dtype=np.float32)
    inv = np.argsort(t2["perm"])
    scale = 1.0 / t2["w"][t2["perm"]]
    for core in range(CORES):
        o = np.asarray(results[core]["out"], np.float32)  # [SHARD, 128]
        o = (o * scale[None, :].astype(np.float32))[:, inv]
        nmc = nodemap[core]
        sel = nmc >= 0
        out[nmc[sel]] = o[sel]
    return out


_CACHE = {}


def kernel(**inputs) -> np.ndarray:
    from concourse.bass_utils import run_bass_kernel_spmd

    edge_index = np.asarray(inputs["edge_index"])
    key = hash(edge_index.tobytes())
    if key not in _CACHE:
        pp = preprocess(edge_index, FULL)
        t1, t2 = host_transforms(inputs)
        nc = build_program(pp, t1["Pp"], t2["Pp"], rep=1)
        _CACHE[key] = (pp, nc)
    pp, nc = _CACHE[key]
    in_maps, (t1, t2) = make_inputs(inputs["node_fts"], inputs, pp)
    res = run_bass_kernel_spmd(nc, in_maps, list(range(CORES)))
    return postprocess(res.results, pp, t2)


if __name__ == "__main__":
    import reference

    inputs = {k: np.asarray(v) for k, v in reference.setup_inputs().items()}
    pp = preprocess(inputs["edge_index"], FULL)
    K = pp["K"]
    tot = int(K.sum()) * 128
    print(f"slots/core {tot} vs {FULL.E//CORES} -> overhead {tot/(FULL.E/CORES)-1:+.1%}")
    import jax

    with jax.default_device(jax.devices("cpu")[0]):
        exp = np.asarray(reference.reference(**inputs))
    got = emulate(inputs["node_fts"], inputs, pp)
    err = np.linalg.norm(got - exp) / np.linalg.norm(exp)
    print(f"numpy emulator rel err: {err:.2e}")



# revision 27
# speedup vs baseline: 1.7366x; 1.7366x over previous
"""GATv2 (2-layer, heads=1) on 8 trn2 NeuronCores — self-contained kernel.

v2 design (batched, partition-major table, 2 gather windows):
- Nodes dealt to 8 cores by in-degree rank; each core owns SHARD=6272 node
  slots (49 chunks x 128 lanes). The feature table holds ONLY wxl rows
  (w*perm(x@Wl), 256B bf16) in PARTITION-MAJOR order: table row of
  (tile t, lane p) = p*NT + t, so node-phase writes are contiguous per
  partition (1-2KB descriptors) and two int16 gather windows
  ([0,32768) and [17408,50176)) cover all rows.
- Own-shard xr (w*perm(x@Wr)+B) stays in SBUF: layer 1 comes precomputed
  from the host (per-core input), layer 2 is computed on-device from the
  resident transposed activations (49 small matmuls).
- Edge phase per chunk group (<=CAP slot-blocks, 2 gather calls):
  per chunk: v' = g + bcast(xrb) (1-2 vector adds), sign-folded prelu
  (scalar; features >=Pp use prelu(scale=-SLOPE, alpha=1/SLOPE) = -lrelu),
  logit = plain row-sum (1-2 vector reduces), exp (scalar), mask+softmax
  smalls, V = g*bcast(a_n) (in-place), slot-sum via wide identity matmuls
  into PSUM [128,512] + one strided fold-reduce.
- out = fold + (bl'+bias') ; layer-1 rows are PE-transposed into a resident
  x2t tile, AllGathered once, and layer-2 node phase reads shards back.
- Host folds w*perm inverses into next-layer weights / final output.
"""

import numpy as np

SLOPE = 0.2
CORES = 8
GRP = 4    # max chunks per gather group
CAP = 64   # max slot-blocks per gather group
IN = 128
D = 128


class Cfg:
    def __init__(self, N, E, SHARD, W0E, W1B, W1E, W2B):
        self.N, self.E, self.SHARD = N, E, SHARD
        self.NCHUNK = SHARD // 128
        self.NPOS = CORES * SHARD
        self.NT = self.NPOS // 128
        self.W0E, self.W1B, self.W1E, self.W2B = W0E, W1B, W1E, W2B
        self.WB = [0, W1B, W2B]
        self.WE = [W0E, W1E, self.NPOS]
        # reserved pad: core 2, local tile 38, lane 0 (global tile 136)
        # -> table row (136//4)*512 + 0*4 + 0 = 17408, inside all 3 windows
        self.ZCORE = 2
        self.ZPOS = 38 * 128
        self.ZROW = (136 // 4) * 512
        for w in range(3):
            assert self.WB[w] <= self.ZROW < self.WE[w]
            assert self.WE[w] - self.WB[w] <= 32768


FULL = Cfg(N=50000, E=800000, SHARD=6272, W0E=32768, W1B=8704, W1E=41472,
           W2B=17408)


def new_row(pos, cfg):
    """table row id of global position pos (core*SHARD+i): block-interleaved
    (t//4)*512 + lane*4 + t%4 — per-partition-contiguous 4-tile writes while
    keeping ids degree-band-coherent (windows cut by tile group)."""
    t = pos // 128
    lane = pos % 128
    return (t // 4) * 512 + lane * 4 + t % 4


# ----------------------------------------------------------------------------
# host-side graph preprocessing
# ----------------------------------------------------------------------------


def preprocess(edge_index, cfg=FULL):
    src = np.asarray(edge_index[0], dtype=np.int64)
    dst = np.asarray(edge_index[1], dtype=np.int64)
    N, SHARD, NCHUNK, NPOS, NT = cfg.N, cfg.SHARD, cfg.NCHUNK, cfg.NPOS, cfg.NT
    deg = np.bincount(dst, minlength=N)

    order = np.argsort(-deg, kind="stable")
    core_of = np.empty(N, dtype=np.int64)
    core_of[order] = np.arange(N) % CORES

    # positions available per core (ZCORE skips the reserved zero-pad slot)
    def positions(c, n):
        if c == cfg.ZCORE:
            p = np.concatenate([np.arange(cfg.ZPOS), np.arange(cfg.ZPOS + 1, SHARD)])
            return p[:n]
        return np.arange(n)

    def assign_rows(key1, key2):
        row_of = np.empty(N, dtype=np.int64)
        nodemap = np.full((CORES, SHARD), -1, dtype=np.int64)
        for c in range(CORES):
            nodes = np.where(core_of == c)[0]
            k = np.lexsort((-key2[nodes], -key1[nodes]))
            nodes = nodes[k]
            pos = positions(c, len(nodes))
            row_of[nodes] = c * SHARD + pos
            nodemap[c, pos] = nodes
        return row_of, nodemap

    row_of, nodemap = assign_rows(deg, np.zeros(N, dtype=np.int64))
    for _ in range(2):
        srows = new_row(row_of[src], cfg)
        m0 = np.bincount(dst[srows < cfg.W1B], minlength=N)
        m2 = np.bincount(dst[srows >= cfg.W1E], minlength=N)
        row_of, nodemap = assign_rows(deg, m0 - m2)

    src_nrows = new_row(row_of[src], cfg)
    dst_rows = row_of[dst]

    eorder = np.lexsort((src_nrows, dst_rows))
    s_sorted = src_nrows[eorder]
    d_sorted = dst_rows[eorder]
    starts = np.searchsorted(d_sorted, np.arange(NPOS))
    ends = np.searchsorted(d_sorted, np.arange(NPOS) + 1)

    lanes_chunk = (np.arange(NPOS) % SHARD) // 128

    # per-chunk window profile (K0, K1, K2) via 2D search, shared across cores
    K0 = np.zeros(NCHUNK, dtype=np.int64)
    K1 = np.zeros(NCHUNK, dtype=np.int64)
    K2 = np.zeros(NCHUNK, dtype=np.int64)
    for c in range(NCHUNK):
        lanes = np.where(lanes_chunk == c)[0]
        lists = [s_sorted[starts[p] : ends[p]] for p in lanes]
        degl = np.array([len(sl) for sl in lists])
        n_lt_w0e = np.array([np.searchsorted(sl, cfg.W0E) for sl in lists])
        n_lt_w1e = np.array([np.searchsorted(sl, cfg.W1E) for sl in lists])
        n_lt_w1b = np.array([np.searchsorted(sl, cfg.W1B) for sl in lists])
        n_lt_w2b = np.array([np.searchsorted(sl, cfg.W2B) for sl in lists])
        best = None
        for k0 in range(max(1, int(n_lt_w1b.max())), int(n_lt_w0e.max()) + 2):
            t0 = np.minimum(n_lt_w0e, k0)
            m1 = np.maximum(n_lt_w2b - t0, 0)
            for k1 in range(max(1, int(m1.max())), int((n_lt_w1e - t0).max()) + 2):
                t1 = np.minimum(n_lt_w1e - t0, k1)
                k2 = max(1, int((degl - t0 - t1).max()))
                if best is None or k0 + k1 + k2 < best[0]:
                    best = (k0 + k1 + k2, k0, k1, k2)
        K0[c], K1[c], K2[c] = best[1], best[2], best[3]

    K = K0 + K1 + K2
    koff = np.concatenate([[0], np.cumsum(K)])
    TK = int(koff[-1])

    # idx grids (window-relative values) + mask; per chunk: [K0 | K1 | K2]
    zrel = [cfg.ZROW - cfg.WB[w] for w in range(3)]
    idx_rows = np.zeros((CORES, TK, 128), dtype=np.int64)
    mask = np.zeros((CORES, TK, 128), dtype=np.float32)
    for c in range(NCHUNK):
        b = koff[c]
        idx_rows[:, b : b + K0[c], :] = zrel[0]
        idx_rows[:, b + K0[c] : b + K0[c] + K1[c], :] = zrel[1]
        idx_rows[:, b + K0[c] + K1[c] : koff[c + 1], :] = zrel[2]
    for p in range(NPOS):
        e0, e1 = starts[p], ends[p]
        if e1 == e0:
            continue
        core, pos = p // SHARD, p % SHARD
        c, lane = pos // 128, pos % 128
        sl = s_sorted[e0:e1]
        t0 = min(int(np.searchsorted(sl, cfg.W0E)), int(K0[c]))
        t1 = min(int(np.searchsorted(sl, cfg.W1E)) - t0, int(K1[c]))
        t2 = len(sl) - t0 - t1
        assert t2 <= K2[c]
        b = koff[c]
        idx_rows[core, b : b + t0, lane] = sl[:t0] - cfg.WB[0]
        mask[core, b : b + t0, lane] = 1.0
        b1 = koff[c] + K0[c]
        idx_rows[core, b1 : b1 + t1, lane] = sl[t0 : t0 + t1] - cfg.WB[1]
        mask[core, b1 : b1 + t1, lane] = 1.0
        b2 = koff[c] + K0[c] + K1[c]
        idx_rows[core, b2 : b2 + t2, lane] = sl[t0 + t1 :] - cfg.WB[2]
        mask[core, b2 : b2 + t2, lane] = 1.0
    assert (idx_rows >= 0).all()
    assert (idx_rows < 32768).all()

    # greedy chunk groups
    groups = []
    cur, tot = [], 0
    for c in range(NCHUNK):
        k = int(K[c])
        if cur and (tot + k > CAP or len(cur) >= GRP):
            groups.append(cur)
            cur, tot = [], 0
        cur.append(c)
        tot += k
    if cur:
        groups.append(cur)

    return dict(
        row_of=row_of, nodemap=nodemap, K0=K0, K1=K1, K2=K2, K=K, koff=koff,
        idx_rows=idx_rows, mask=mask, groups=groups, cfg=cfg,
    )


def transform_weights(Wl, bl, Wr, br, att, bias, in_perm=None, in_w=None):
    Wl = np.asarray(Wl, np.float64)
    Wr = np.asarray(Wr, np.float64)
    bl = np.asarray(bl, np.float64)
    br = np.asarray(br, np.float64)
    att = np.asarray(att, np.float64)
    bias = np.asarray(bias, np.float64)
    if in_perm is not None:
        scale = 1.0 / in_w[in_perm]
        Wl = Wl[in_perm, :] * scale[:, None]
        Wr = Wr[in_perm, :] * scale[:, None]
    w = np.abs(att)
    pos = np.where(att >= 0)[0]
    neg = np.where(att < 0)[0]
    perm = np.concatenate([pos, neg])

    def colT(W):
        return W[:, perm] * w[perm][None, :]

    def vecT(v):
        return (w * v)[perm]

    return dict(
        Wc=np.concatenate([colT(Wl), colT(Wr)], axis=1),
        Bp=vecT(bl + br), blp=vecT(bl), biasp=vecT(bias),
        perm=perm, w=w, Pp=len(pos),
    )


def host_transforms(params):
    t1 = transform_weights(
        params["Wl1"], params["bl1"], params["Wr1"], params["br1"],
        params["att1"], params["bias1"],
    )
    t2 = transform_weights(
        params["Wl2"], params["bl2"], params["Wr2"], params["br2"],
        params["att2"], params["bias2"], in_perm=t1["perm"], in_w=t1["w"],
    )
    return t1, t2


# ----------------------------------------------------------------------------
# numpy emulator of the device algorithm (validation)
# ----------------------------------------------------------------------------


def emulate(node_fts, params, pp):
    cfg = pp["cfg"]
    t1, t2 = host_transforms(params)
    nodemap = pp["nodemap"]
    nm = nodemap.reshape(-1)
    valid = nm >= 0
    x_rows = np.zeros((cfg.NPOS, IN), dtype=np.float64)
    x_rows[valid] = np.asarray(node_fts, np.float64)[nm[valid]]

    def layer(x_rows, t):
        table = x_rows @ t["Wc"]
        wxl, wxr = table[:, :128], table[:, 128:]
        # reorder wxl into new-row order for gathers
        perm = new_row(np.arange(cfg.NPOS), cfg)
        wxl_n = np.zeros_like(wxl)
        wxl_n[perm] = wxl
        koff, K0, K1 = pp["koff"], pp["K0"], pp["K1"]
        idx_rows, mask = pp["idx_rows"], pp["mask"]
        Pp = t["Pp"]
        BB = t["blp"] + t["biasp"]
        out = np.zeros((cfg.NPOS, 128), dtype=np.float64)
        for core in range(CORES):
            for c in range(cfg.NCHUNK):
                b, e = koff[c], koff[c + 1]
                rows = core * cfg.SHARD + c * 128 + np.arange(128)
                idx = idx_rows[core, b:e, :].copy()
                idx[K0[c] : K0[c] + K1[c], :] += cfg.WB[1]
                idx[K0[c] + K1[c] :, :] += cfg.WB[2]
                m = mask[core, b:e, :]
                xrb = wxr[rows] + t["Bp"][None, :]
                g = wxl_n[idx]                       # [K, lane, d]
                v = g + xrb[None, :, :]
                u = np.where(v > 0, v, SLOPE * v)
                u[:, :, Pp:] = -u[:, :, Pp:]
                logit = u.sum(-1)
                a = np.exp(logit) * m
                s = a.sum(0)
                r = 1.0 / np.maximum(s, 1e-16)
                a_n = a * r[None, :]
                psum = np.einsum("kl,kld->ld", a_n, g)
                out[rows] = psum + BB[None, :]
        return np.maximum(out, 0.0)

    x2 = layer(x_rows, t1)
    out2 = layer(x2, t2)
    un = out2 / t2["w"][t2["perm"]][None, :]
    full = np.zeros((cfg.N, D), dtype=np.float64)
    full[nm[valid]] = un[valid][:, np.argsort(t2["perm"])]
    return full


# ----------------------------------------------------------------------------
# device program
# ----------------------------------------------------------------------------


def wrap_idx(flat):
    """flat int idx list (len % 16 == 0) -> [128, n/16] int16 (16-wrap,
    replicated across the 8 Q7 core blocks)."""
    n = flat.shape[0]
    w = flat.reshape(n // 16, 16).T.astype(np.int16)
    return np.tile(w, (8, 1))


def build_program(pp, Pp1, Pp2, rep=1, debug_out=False, only_layer1=False):
    import concourse.bass as bass
    import concourse.mybir as mybir
    import concourse.tile as tile
    from concourse import bacc

    cfg = pp["cfg"]
    fp32, bf16, i16 = mybir.dt.float32, mybir.dt.bfloat16, mybir.dt.int16
    K0, K1, K2, K, koff = pp["K0"], pp["K1"], pp["K2"], pp["K"], pp["koff"]
    KW = [K0, K1, K2]
    groups = pp["groups"]
    NCHUNK, SHARD, NPOS, NT = cfg.NCHUNK, cfg.SHARD, cfg.NPOS, cfg.NT
    TK = int(koff[-1])
    ADD, MULT = mybir.AluOpType.add, mybir.AluOpType.mult

    gsz = [tuple(int(sum(kw[c] for c in chs)) for kw in KW) for chs in groups]
    icols = [sum(g[w] * 8 for g in gsz) for w in range(3)]

    nc = bacc.Bacc(
        "TRN2", target_bir_lowering=False, debug=False, num_devices=CORES,
        num_swdge_queues=4,
    )
    xT = nc.dram_tensor("xT", [128, NPOS], bf16, kind="ExternalInput").ap()
    xrb0 = nc.dram_tensor("xrb0", [128, SHARD], bf16, kind="ExternalInput").ap()
    W1c = nc.dram_tensor("W1c", [128, 256], bf16, kind="ExternalInput").ap()
    W2c = nc.dram_tensor("W2c", [128, 256], bf16, kind="ExternalInput").ap()
    # consts: [identity | B2 | BB1 | BB2] each [128,128]
    cons = nc.dram_tensor("cons", [128, 4 * 128], bf16, kind="ExternalInput").ap()
    idx0 = nc.dram_tensor("idx0", [128, icols[0]], i16, kind="ExternalInput").ap()
    idx1 = nc.dram_tensor("idx1", [128, icols[1]], i16, kind="ExternalInput").ap()
    idx2 = nc.dram_tensor("idx2", [128, icols[2]], i16, kind="ExternalInput").ap()
    maskt = nc.dram_tensor("maskt", [128, TK], bf16, kind="ExternalInput").ap()
    table = nc.dram_tensor("table", [NPOS, 128], bf16)
    x2t_d = nc.dram_tensor("x2t_d", [128, SHARD], bf16)
    ag = nc.dram_tensor("ag", [CORES * 128, SHARD], bf16, addr_space="Shared")
    out_ext = nc.dram_tensor("out", [SHARD, 128], fp32, kind="ExternalOutput").ap()
    if debug_out:
        dbg_table = nc.dram_tensor(
            "dbg_table", [NPOS, 128], bf16, kind="ExternalOutput").ap()
        dbg_x2t = nc.dram_tensor(
            "dbg_x2t", [128, SHARD], bf16, kind="ExternalOutput").ap()

    # row (t, p) lives at (t//4)*512 + p*4 + t%4  ->  [p, q, j, c] view
    tab4 = table.rearrange("(q p j) c -> p q j c", p=128, j=4)
    NQ = NPOS // 512  # 98 global 4-tile groups

    with tile.TileContext(nc) as tc:
        with (
            tc.tile_pool(name="res", bufs=1) as res,
            tc.tile_pool(name="nsb", bufs=3) as nsb,
            tc.tile_pool(name="gsb", bufs=2) as gsb,
            tc.tile_pool(name="usb", bufs=2) as usb,
            tc.tile_pool(name="csb", bufs=3) as csb,
            tc.tile_pool(name="ps", bufs=2, space="PSUM") as ps,
            tc.tile_pool(name="ps2", bufs=2, space="PSUM") as ps2,
            tc.tile_pool(name="ps3", bufs=2, space="PSUM") as ps3,
        ):
            i0_sb = res.tile([128, icols[0]], i16, tag="i0")
            i1_sb = res.tile([128, icols[1]], i16, tag="i1")
            i2_sb = res.tile([128, icols[2]], i16, tag="i2")
            mk_sb = res.tile([128, TK], bf16, tag="mk")
            co_sb = res.tile([128, 4 * 128], bf16, tag="co")
            w1_sb = res.tile([128, 256], bf16, tag="w1")
            w2_sb = res.tile([128, 256], bf16, tag="w2")
            xrb1_sb = res.tile([128, SHARD], bf16, tag="xrb1")
            xrb2_sb = res.tile([128, SHARD], bf16, tag="xrb2")
            x2t_sb = res.tile([128, SHARD], bf16, tag="x2t")
            consf = res.tile([128, 2 * 128], fp32, tag="cof")
            nc.sync.dma_start(out=i0_sb[:], in_=idx0[:])
            nc.sync.dma_start(out=i1_sb[:], in_=idx1[:])
            nc.sync.dma_start(out=i2_sb[:], in_=idx2[:])
            nc.sync.dma_start(out=mk_sb[:], in_=maskt[:])
            nc.sync.dma_start(out=co_sb[:], in_=cons[:])
            nc.sync.dma_start(out=w1_sb[:], in_=W1c[:])
            nc.sync.dma_start(out=w2_sb[:], in_=W2c[:])
            nc.sync.dma_start(out=xrb1_sb[:], in_=xrb0[:])
            ident = co_sb[:, 0:128]
            B2_rep = co_sb[:, 128:256]
            # fp32 copies of BB1 / BB2
            nc.scalar.copy(consf[:, 0:128], co_sb[:, 256:384])
            nc.scalar.copy(consf[:, 128:256], co_sb[:, 384:512])

            def node_phase(layer):
                w_sb = w1_sb if layer == 1 else w2_sb
                for q in range(NQ):
                    t0 = q * 4
                    lhs = nsb.tile([128, 512], bf16, tag="lhs")
                    if layer == 1:
                        nc.sync.dma_start(
                            out=lhs[:], in_=xT[:, t0 * 128 : (t0 + 4) * 128])
                    else:
                        # owner runs within the 4-tile group
                        j = 0
                        while j < 4:
                            o = (t0 + j) // NCHUNK
                            nr = min(4 - j, (o + 1) * NCHUNK - (t0 + j))
                            tl = (t0 + j) % NCHUNK
                            nc.sync.dma_start(
                                out=lhs[:, j * 128 : (j + nr) * 128],
                                in_=ag[o * 128 : (o + 1) * 128,
                                       tl * 128 : (tl + nr) * 128],
                            )
                            j += nr
                    pt = ps.tile([128, 1024], fp32, tag="np")
                    for j in range(4):
                        nc.tensor.matmul(
                            pt[:, j * 256 : (j + 1) * 256],
                            lhsT=lhs[:, j * 128 : (j + 1) * 128],
                            rhs=w_sb[:],
                            start=True, stop=True,
                        )
                    rows = nsb.tile([128, 512], bf16, tag="rows")
                    src = pt[:].rearrange("p (j c) -> p j c", c=256)[:, :, 0:128]
                    if q % 2 == 0:
                        nc.vector.tensor_copy(
                            rows[:].rearrange("p (j c) -> p j c", c=128), src)
                    else:
                        nc.scalar.copy(
                            rows[:].rearrange("p (j c) -> p j c", c=128), src)
                    nc.sync.dma_start(
                        out=tab4[:, q, :, :],
                        in_=rows[:].rearrange("p (j c) -> p j c", c=128),
                    )

            def xrb2_compute():
                # xrb(layer2) = x2t.T @ Wr2 + B2, chunks of 4
                for q in range(0, NCHUNK, 4):
                    nq = min(4, NCHUNK - q)
                    xp = ps.tile([128, 1024], fp32, tag="np")
                    for j in range(nq):
                        c = q + j
                        nc.tensor.matmul(
                            xp[:, j * 128 : (j + 1) * 128],
                            lhsT=x2t_sb[:, c * 128 : (c + 1) * 128],
                            rhs=w2_sb[:, 128:256],
                            start=True, stop=True,
                        )
                    dst = xrb2_sb[:].rearrange("p (c f) -> p c f", f=128)[
                        :, q : q + nq, :]
                    nc.vector.tensor_tensor(
                        out=dst,
                        in0=xp[:].rearrange("p (c f) -> p c f", f=128)[:, 0:nq, :],
                        in1=B2_rep.unsqueeze(1).to_broadcast([128, nq, 128]),
                        op=ADD,
                    )

            def edge_phase(layer):
                Pp = Pp1 if layer == 1 else Pp2
                BBf = consf[:, 0:128] if layer == 1 else consf[:, 128:256]
                isbs = [i0_sb, i1_sb, i2_sb]
                xrb_sb = xrb1_sb if layer == 1 else xrb2_sb
                offs = [0, 0, 0]
                for gi, chs in enumerate(groups):
                    nbw = gsz[gi]
                    kg = sum(nbw)
                    wbase = [0, nbw[0], nbw[0] + nbw[1]]
                    gt = gsb.tile([128, kg * 128], bf16, tag="g")
                    g3 = gt[:].rearrange("p (b r) -> p b r", r=128)
                    ut = usb.tile([128, kg * 128], bf16, tag="u")
                    u3 = ut[:].rearrange("p (b r) -> p b r", r=128)
                    for w in range(3):
                        nb = nbw[w]
                        if nb == 0:
                            continue
                        nc.gpsimd.dma_gather(
                            out_ap=g3[:, wbase[w] : wbase[w] + nb, :],
                            in_ap=table[cfg.WB[w] :, :],
                            idxs_ap=isbs[w][:, offs[w] : offs[w] + nb * 8],
                            num_idxs=nb * 128,
                            num_idxs_reg=nb * 128,
                            elem_size=128,
                            single_packet=False,
                            queue_num=(gi * 3 + w) % 4,
                        )
                        offs[w] += nb * 8

                    apos = list(wbase)
                    for ci, c in enumerate(chs):
                        kwc = [int(kw[c]) for kw in KW]
                        kc = sum(kwc)
                        runs = list(zip(apos, kwc))
                        xrb_c = xrb_sb[:, c * 128 : (c + 1) * 128].unsqueeze(1)
                        for b0, nb_ in runs:
                            if nb_ == 0:
                                continue
                            nc.vector.tensor_tensor(
                                out=u3[:, b0 : b0 + nb_, :],
                                in0=g3[:, b0 : b0 + nb_, :],
                                in1=xrb_c.to_broadcast([128, nb_, 128]),
                                op=ADD,
                            )
                            nc.scalar.activation(
                                out=u3[:, b0 : b0 + nb_, 0:Pp],
                                in_=u3[:, b0 : b0 + nb_, 0:Pp],
                                func=mybir.ActivationFunctionType.Prelu,
                                alpha=SLOPE,
                            )
                            if Pp < 128:
                                nc.scalar.activation(
                                    out=u3[:, b0 : b0 + nb_, Pp:128],
                                    in_=u3[:, b0 : b0 + nb_, Pp:128],
                                    func=mybir.ActivationFunctionType.Prelu,
                                    scale=-SLOPE,
                                    alpha=1.0 / SLOPE,
                                )
                        lg = csb.tile([128, kc], fp32, tag="lg")
                        s0 = 0
                        for b0, nb_ in runs:
                            if nb_ == 0:
                                continue
                            nc.vector.tensor_reduce(
                                out=lg[:, s0 : s0 + nb_],
                                in_=u3[:, b0 : b0 + nb_, :],
                                axis=mybir.AxisListType.X, op=ADD,
                            )
                            s0 += nb_
                        av = csb.tile([128, kc], fp32, tag="av")
                        nc.scalar.activation(
                            out=av[:], in_=lg[:],
                            func=mybir.ActivationFunctionType.Exp,
                        )
                        nc.vector.tensor_tensor(
                            out=av[:], in0=av[:],
                            in1=mk_sb[:, koff[c] : koff[c] + kc], op=MULT,
                        )
                        sv = csb.tile([128, 3], fp32, tag="sv")
                        nc.vector.tensor_reduce(
                            out=sv[:, 0:1], in_=av[:],
                            axis=mybir.AxisListType.X, op=ADD,
                        )
                        nc.vector.tensor_scalar_max(sv[:, 1:2], sv[:, 0:1], 1e-16)
                        nc.vector.reciprocal(sv[:, 2:3], sv[:, 1:2])
                        an = csb.tile([128, kc], fp32, tag="an")
                        nc.vector.tensor_scalar(
                            out=an[:], in0=av[:], scalar1=sv[:, 2:3], scalar2=None,
                            op0=MULT,
                        )
                        # V = g * bcast(an), in place
                        s0 = 0
                        for b0, nb_ in runs:
                            if nb_ == 0:
                                continue
                            nc.vector.tensor_tensor(
                                out=g3[:, b0 : b0 + nb_, :],
                                in0=g3[:, b0 : b0 + nb_, :],
                                in1=an[:, s0 : s0 + nb_].unsqueeze(2)
                                    .to_broadcast([128, nb_, 128]),
                                op=MULT,
                            )
                            s0 += nb_
                        # slot-sum: wide identity matmuls into PSUM + fold
                        wides = []
                        for b0, nb_ in runs:
                            for bb in range(b0, b0 + nb_, 4):
                                wides.append((bb, min(4, b0 + nb_ - bb)))
                        qmax = max(w4 for _, w4 in wides)
                        fp = ps2.tile([128, 512], fp32, tag="fold")
                        for si, (bb, w4) in enumerate(wides):
                            nc.tensor.matmul(
                                fp[:, 0 : w4 * 128],
                                lhsT=ident,
                                rhs=gt[:, bb * 128 : (bb + w4) * 128],
                                start=(si == 0), stop=(si == len(wides) - 1),
                                skip_group_check=True,
                            )
                        of = csb.tile([128, 128], fp32, tag="of")
                        nc.vector.tensor_reduce(
                            out=of[:],
                            in_=fp[:].rearrange("p (q r) -> p q r", r=128)[
                                :, 0:qmax, :].transpose([0, 2, 1]),
                            axis=mybir.AxisListType.X, op=ADD,
                        )
                        if layer == 1:
                            x2row = csb.tile([128, 128], bf16, tag="x2r")
                            nc.vector.tensor_tensor(
                                out=x2row[:], in0=of[:], in1=BBf, op=ADD)
                            tp = ps3.tile([128, 128], bf16, tag="tr")
                            nc.tensor.matmul(
                                tp[:], lhsT=x2row[:], rhs=ident,
                                is_transpose=True, start=True, stop=True,
                            )
                            nc.scalar.activation(
                                out=x2t_sb[:, c * 128 : (c + 1) * 128],
                                in_=tp[:],
                                func=mybir.ActivationFunctionType.Relu,
                            )
                        else:
                            orow = csb.tile([128, 128], fp32, tag="orow")
                            nc.vector.tensor_tensor(
                                out=orow[:], in0=of[:], in1=BBf, op=ADD)
                            orow2 = csb.tile([128, 128], fp32, tag="orow2")
                            nc.scalar.activation(
                                out=orow2[:], in_=orow[:],
                                func=mybir.ActivationFunctionType.Relu,
                            )
                            nc.sync.dma_start(
                                out=out_ext[c * 128 : (c + 1) * 128, :],
                                in_=orow2[:],
                            )
                        for w in range(3):
                            apos[w] += kwc[w]

            for _ in range(rep):
                node_phase(1)
                edge_phase(1)
                if debug_out:
                    nc.sync.dma_start(out=dbg_table[:], in_=table[:])
                    nc.sync.dma_start(out=dbg_x2t[:], in_=x2t_sb[:])
                if only_layer1:
                    continue
                nc.sync.dma_start(out=x2t_d[:], in_=x2t_sb[:])
                nc.gpsimd.collective_compute(
                    "AllGather",
                    mybir.AluOpType.bypass,
                    replica_groups=[list(range(CORES))],
                    ins=[x2t_d[:]],
                    outs=[ag[:]],
                )
                xrb2_compute()
                node_phase(2)
                edge_phase(2)

    nc.compile()
    return nc


# ----------------------------------------------------------------------------
# host input packing + entry point
# ----------------------------------------------------------------------------


def make_inputs(node_fts, params, pp):
    import ml_dtypes

    bf = ml_dtypes.bfloat16
    cfg = pp["cfg"]
    t1, t2 = host_transforms(params)
    nodemap = pp["nodemap"]
    nm = nodemap.reshape(-1)
    valid = nm >= 0
    x_rows = np.zeros((cfg.NPOS, IN), dtype=np.float32)
    x_rows[valid] = np.asarray(node_fts, np.float32)[nm[valid]]
    xT = np.ascontiguousarray(x_rows.T).astype(bf)

    def rep(v):
        return np.tile(np.asarray(v, np.float32).astype(bf)[None, :], (128, 1))

    cons = np.concatenate(
        [np.eye(128, dtype=np.float32).astype(bf),
         rep(t2["Bp"]),  # B2
         rep(t1["blp"] + t1["biasp"]),  # BB1
         rep(t2["blp"] + t2["biasp"])], axis=1)  # BB2

    K0, K1, K2, koff = pp["K0"], pp["K1"], pp["K2"], pp["koff"]
    groups = pp["groups"]
    idx_rows, mask = pp["idx_rows"], pp["mask"]
    SHARD = cfg.SHARD

    # per-core xrb (layer1): (own_x @ Wc1[:,128:]) + B1, laid out [128, SHARD]
    Wr1c = np.asarray(t1["Wc"], np.float64)[:, 128:]
    B1 = np.asarray(t1["Bp"], np.float64)

    in_maps = []
    for core in range(CORES):
        xr = x_rows[core * SHARD : (core + 1) * SHARD].astype(np.float64) @ Wr1c
        xr = xr + B1[None, :]
        xrb0 = np.ascontiguousarray(
            xr.reshape(cfg.NCHUNK, 128, 128).transpose(1, 0, 2).reshape(128, -1)
        ).astype(np.float32).astype(bf)
        i0l, i1l, i2l = [], [], []
        for chs in groups:
            f0 = [idx_rows[core, koff[c] : koff[c] + K0[c], :] for c in chs]
            f1 = [idx_rows[core, koff[c] + K0[c] : koff[c] + K0[c] + K1[c], :]
                  for c in chs]
            f2 = [idx_rows[core, koff[c] + K0[c] + K1[c] : koff[c + 1], :]
                  for c in chs]
            i0l.append(wrap_idx(np.concatenate(f0).ravel()))
            i1l.append(wrap_idx(np.concatenate(f1).ravel()))
            i2l.append(wrap_idx(np.concatenate(f2).ravel()))
        in_maps.append(
            {
                "xT": xT,
                "xrb0": xrb0,
                "W1c": np.asarray(t1["Wc"], np.float32).astype(bf),
                "W2c": np.asarray(t2["Wc"], np.float32).astype(bf),
                "cons": cons.astype(bf),
                "idx0": np.concatenate(i0l, axis=1),
                "idx1": np.concatenate(i1l, axis=1),
                "idx2": np.concatenate(i2l, axis=1),
                "maskt": np.ascontiguousarray(mask[core].T).astype(bf),
            }
        )
    return in_maps, (t1, t2)


def postprocess(results, pp, t2):
    cfg = pp["cfg"]
    nodemap = pp["nodemap"]
    out = np.zeros((cfg.N, D), dtype=np.float32)
    inv = np.argsort(t2["perm"])
    scale = 1.0 / t2["w"][t2["perm"]]
    for core in range(CORES):
        o = np.asarray(results[core]["out"], np.float32)  # [SHARD, 128]
        o = (o * scale[None, :].astype(np.float32))[:, inv]
        nmc = nodemap[core]
        sel = nmc >= 0
        out[nmc[sel]] = o[sel]
    return out


_CACHE = {}


def kernel(**inputs) -> np.ndarray:
    from concourse.bass_utils import run_bass_kernel_spmd

    edge_index = np.asarray(inputs["edge_index"])
    key = hash(edge_index.tobytes())
    if key not in _CACHE:
        pp = preprocess(edge_index, FULL)
        t1, t2 = host_transforms(inputs)
        nc = build_program(pp, t1["Pp"], t2["Pp"], rep=1)
        _CACHE[key] = (pp, nc)
    pp, nc = _CACHE[key]
    in_maps, (t1, t2) = make_inputs(inputs["node_fts"], inputs, pp)
    res = run_bass_kernel_spmd(nc, in_maps, list(range(CORES)))
    return postprocess(res.results, pp, t2)


if __name__ == "__main__":
    import reference

    inputs = {k: np.asarray(v) for k, v in reference.setup_inputs().items()}
    pp = preprocess(inputs["edge_index"], FULL)
    K = pp["K"]
    tot = int(K.sum()) * 128
    print(f"slots/core {tot} vs {FULL.E//CORES} -> overhead {tot/(FULL.E/CORES)-1:+.1%}")
    print(f"groups: {len(pp['groups'])}")
    import jax

    with jax.default_device(jax.devices("cpu")[0]):
        exp = np.asarray(reference.reference(**inputs))
    got = emulate(inputs["node_fts"], inputs, pp)
    err = np.linalg.norm(got - exp) / np.linalg.norm(exp)
    print(f"numpy emulator rel err: {err:.2e}")
